# revision 8
# baseline (speedup 1.0000x reference)
"""BipartiteGCN message-passing kernel for 8 TRN2 NeuronCores.

Math:  out = D_c^{-1/2} A^T D_r^{-1/2} (x @ W) + b
where A[s, d] = multiplicity of edge (gene s, drug d), s, d in [0, 4000).

Strategy (gene-shard + split ReduceScatter, no gather/scatter DMA):
  - Core c owns gene (src) window [512c, 512c+512).  Its edges (src in
    window) are binned by (dst window 0..31, gene subwindow 0..3) into
    128-edge chunks (host-side layout only; all arithmetic on device).
  - xw_c = x_c @ W computed locally in bf16 (x/W converted on the
    Activation engine, quarter-pipelined with their DMAs), rows scaled
    by f = rsqrt(row_deg).  Degrees are counted on device from
    host-binned edge-id tensors (compare-vs-iota + reduce) — no degree
    collective; gene-sharding makes row_deg core-local.
  - A_dw count blocks ([128 gene x 4x128 dst]) built with one-hot x
    one-hot PE matmuls; f16 one-hots from compare-vs-iota, alternating
    between DVE and Pool (GpSimd), with half-B one-hots emitted early
    so both engines stay saturated.
  - partial[dst, oc] = A_dw^T @ xw_f per dst window (4 accumulating
    bf16 matmuls), stored f16 to DRAM in 2-window stages.
  - Two ReduceScatters (f16, 0.25 MB out each) over the dst halves
    (dwl 0,1 | 2,3); the first fully overlaps the second half's
    compute.  Collectives live alone at the end of the Pool queue so
    they cannot stall the worker engines.
  - Post-scatter g = rsqrt(col_deg) scaling and the bias add run ON
    THE PE (psum = diag(g)^T @ rs + ones^T @ bias): the scheduling
    pass under-costs collectives, so any RS-dependent work placed on
    DVE/ACT/Pool poisons the vector-clock fences of the second-half
    pipeline and stalls it.  A strict tile barrier pins the post
    section after the pipeline.
  - Core c ends with dst rows [512c, 512c+512); host reassembles.
"""

import sys

if "/opt/trn_rl_repo" not in sys.path:
    sys.path.insert(0, "/opt/trn_rl_repo")

import numpy as np

import concourse.bass as bass  # noqa: F401
import concourse.mybir as mybir
from concourse import bacc, tile

CORES = 8
GSH = 512               # genes per core
GW = 4                  # gene subwindows (128) per core
DWT = 32                # dst windows of 128 (4096 padded drugs)
ND = 4000
IC = 1024
OC = 512

F32 = mybir.dt.float32
F32R = mybir.dt.float32r
F16 = mybir.dt.float16
BF16 = mybir.dt.bfloat16

# Static layout knobs (make_in_maps bumps them if the distribution is
# pathological; kernel() then rebuilds).
_WCH = 3                # 128-edge chunks per (dst window, gene subwindow) cell
_CELLCH = None          # per-cell chunk counts (max over cores); set by make_in_maps
_PADR = 128             # row-degree bin slots per (partition, gene subwindow)
_PADC = 128             # col-degree bin slots per (partition, own dst window)

XW_AT = 7               # dw order index at which the xw GEMM is emitted
PO_AT = 12              # dw order index at which po draining starts
PO_LAG = 1              # steady-state po lag behind A-build

# dst-window order: each shard's windows 0/1 form half A, 2/3 half B;
# the two halves are reduce-scattered separately so RS0 overlaps half B
DWORDER = [4 * c + j for c in range(CORES) for j in (0, 1)] + \
          [4 * c + j for c in range(CORES) for j in (2, 3)]


def build_nc(wch=None, padr=None, padc=None, cellch=None):
    wch = _WCH if wch is None else wch
    padr = _PADR if padr is None else padr
    padc = _PADC if padc is None else padc
    cellch = (_CELLCH if _CELLCH is not None
              else [wch] * (DWT * GW)) if cellch is None else cellch
    cellbase = [0] * (DWT * GW)
    acc = 0
    for ci in range(DWT * GW):
        cellbase[ci] = acc
        acc += cellch[ci]
    nch = acc

    nc = bacc.Bacc(
        None,
        target_bir_lowering=False,
        debug=False,
        num_devices=CORES,
    )

    xT = nc.dram_tensor("xT", [128, 8, GSH], F32, kind="ExternalInput")
    w = nc.dram_tensor("w", [128, 8, OC], F32, kind="ExternalInput")
    brep = nc.dram_tensor("brep", [128, OC], F32, kind="ExternalInput")
    i128 = nc.dram_tensor("i128", [128, 128], F16, kind="ExternalInput")
    pcol = nc.dram_tensor("pcol", [128, 1], F32, kind="ExternalInput")
    sloc = nc.dram_tensor("sloc", [128, nch], F32, kind="ExternalInput")
    dloc = nc.dram_tensor("dloc", [128, nch], F32, kind="ExternalInput")
    ssrc = nc.dram_tensor("ssrc", [128, GW * padr], F16, kind="ExternalInput")
    sdst = nc.dram_tensor("sdst", [128, 4 * padc], F16, kind="ExternalInput")
    out = nc.dram_tensor("out", [128, 4, OC], F32, kind="ExternalOutput")

    prts = [nc.dram_tensor(f"prt{h}", [CORES, 128, 2, OC], F16) for h in (0, 1)]
    rsouts = [nc.dram_tensor(f"rsout{h}", [128, 2, OC], F16) for h in (0, 1)]

    with tile.TileContext(nc) as tc:
        with (
            tc.tile_pool(name="const", bufs=1) as cpool,
            tc.tile_pool(name="work", bufs=2) as wpool,
            tc.tile_pool(name="ohP", bufs=2) as poolP,
            tc.tile_pool(name="ohD", bufs=2) as poolD,
            tc.tile_pool(name="adwp", bufs=2) as padw,
            tc.tile_pool(name="psum", bufs=2, space="PSUM") as ppool,
        ):
            # ---- inputs (order = DMA priority) ----
            i128_sb = cpool.tile([128, 128], F16)
            nc.sync.dma_start(i128_sb[:], i128[:])
            sloc_sb = cpool.tile([128, nch], F32)
            nc.sync.dma_start(sloc_sb[:], sloc[:])
            dloc_sb = cpool.tile([128, nch], F32)
            nc.sync.dma_start(dloc_sb[:], dloc[:])
            pcol_sb = cpool.tile([128, 1], F32)
            nc.sync.dma_start(pcol_sb[:], pcol[:])
            ssrc_sb = cpool.tile([128, GW * padr], F16)
            nc.sync.dma_start(ssrc_sb[:], ssrc[:])
            sdst_sb = cpool.tile([128, 4 * padc], F16)
            nc.sync.dma_start(sdst_sb[:], sdst[:])
            xT_sb = cpool.tile([128, 8 * GSH], F32)
            w_sb = cpool.tile([128, 8 * OC], F32)
            for q in range(4):
                nc.sync.dma_start(
                    xT_sb[:, 2 * q * GSH:2 * (q + 1) * GSH], xT[:, 2 * q:2 * q + 2, :]
                )
                nc.sync.dma_start(
                    w_sb[:, 2 * q * OC:2 * (q + 1) * OC], w[:, 2 * q:2 * q + 2, :]
                )
            bias_sb = cpool.tile([128, OC], F32)
            nc.sync.dma_start(bias_sb[:], brep[:])
            xTb = cpool.tile([128, 8 * GSH], BF16)
            wb = cpool.tile([128, 8 * OC], BF16)
            for q in range(4):
                nc.scalar.copy(
                    xTb[:, 2 * q * GSH:2 * (q + 1) * GSH],
                    xT_sb[:, 2 * q * GSH:2 * (q + 1) * GSH],
                )
                nc.scalar.copy(
                    wb[:, 2 * q * OC:2 * (q + 1) * OC],
                    w_sb[:, 2 * q * OC:2 * (q + 1) * OC],
                )
            bias16 = cpool.tile([1, OC], F16)
            nc.scalar.copy(bias16[:], bias_sb[0:1, :])

            # ---- PE warmup past the p-state ramp while DMAs stream
            wrm = cpool.tile([128, OC], F16)
            nc.gpsimd.memset(wrm[:], 0.0)
            pwarm = ppool.tile([128, OC], F32, tag="pxw", bufs=4)
            for i in range(10):
                nc.tensor.matmul(
                    pwarm[:, 0:256], wrm[:, 0:128], wrm[:, 0:256],
                    start=(i == 0), stop=(i == 9),
                )

            # ---- degree -> rsqrt scale helper (counts via compare+reduce)
            def deg_scale(src_sb, nwin, pad):
                eqt = cpool.tile([128, nwin * pad], F16)
                for wi in range(nwin):
                    nc.vector.tensor_scalar(
                        out=eqt[:, wi * pad:(wi + 1) * pad],
                        in0=src_sb[:, wi * pad:(wi + 1) * pad],
                        scalar1=pcol_sb[:, 0:1], scalar2=None,
                        op0=mybir.AluOpType.is_equal,
                    )
                deg = cpool.tile([128, nwin], F32)
                for wi in range(nwin):
                    nc.vector.reduce_sum(
                        deg[:, wi:wi + 1], eqt[:, wi * pad:(wi + 1) * pad],
                        axis=mybir.AxisListType.X,
                    )
                t1 = cpool.tile([128, nwin], F32)
                nc.vector.tensor_scalar(
                    out=t1[:], in0=deg[:], scalar1=1.0, scalar2=None,
                    op0=mybir.AluOpType.max,
                )
                nc.scalar.sqrt(t1[:], t1[:])
                nc.vector.reciprocal(t1[:], t1[:])
                msk = cpool.tile([128, nwin], F32)
                nc.vector.tensor_scalar(
                    out=msk[:], in0=deg[:], scalar1=0.5, scalar2=None,
                    op0=mybir.AluOpType.is_gt,
                )
                sc = cpool.tile([128, nwin], F32)
                nc.vector.tensor_tensor(
                    out=sc[:], in0=t1[:], in1=msk[:], op=mybir.AluOpType.mult
                )
                return sc

            f_sb = None
            dgt = []
            onesr = cpool.tile([1, 128], F16)
            nc.gpsimd.memset(onesr[:], 1.0)

            def emit_g_diag():
                g_sb = deg_scale(sdst_sb, 4, padc)    # per own-window dst
                for j in range(4):
                    d_t = cpool.tile([128, 128], F16, name=f"dgt{j}")
                    nc.vector.tensor_scalar(
                        out=d_t[:], in0=i128_sb[:],
                        scalar1=pcol_sb[:, 0:1], scalar2=g_sb[:, j:j + 1],
                        op0=mybir.AluOpType.is_equal, op1=mybir.AluOpType.mult,
                    )
                    dgt.append(d_t)

            # ---- emitted later: xw = x_c @ W with f-scaled rows
            xwf = [cpool.tile([128, OC], BF16, name=f"xwf{gt}") for gt in range(GW)]
            pbs = [None] * GW

            def emit_xw(q):
                for gt in range(GW):
                    if q == 0:
                        pbs[gt] = ppool.tile([128, OC], F32, tag="pxw", bufs=4,
                                             name=f"pb{gt}")
                    for kt in range(2 * q, 2 * q + 2):
                        nc.tensor.matmul(
                            pbs[gt][:],
                            xTb[:, kt * GSH + gt * 128:kt * GSH + (gt + 1) * 128],
                            wb[:, kt * OC:(kt + 1) * OC],
                            start=(kt == 0),
                            stop=(kt == 7),
                        )
                    if q == 3:
                        nc.scalar.mul(xwf[gt][:], pbs[gt][:], f_sb[:, gt:gt + 1])

            # ---- per-dst-window pipeline over the interleaved order
            adws = {}
            stage = {}

            ohs = {}

            def emit_ohs(k):
                dw = DWORDER[k]
                for gw in range(GW):
                    ci = dw * GW + gw
                    for i in range(cellch[ci]):
                        c = cellbase[ci] + i
                        on_dve = c % 2 == 0
                        eng = nc.vector if on_dve else nc.gpsimd
                        pool = poolD if on_dve else poolP
                        loh = pool.tile([128, 128], F16, tag="loh",
                                        bufs=96, name=f"loh{c}")
                        roh = pool.tile([128, 128], F16, tag="roh",
                                        bufs=96, name=f"roh{c}")
                        eng.tensor_scalar(
                            out=loh[:], in0=i128_sb[:],
                            scalar1=sloc_sb[:, c:c + 1], scalar2=None,
                            op0=mybir.AluOpType.is_equal,
                        )
                        eng.tensor_scalar(
                            out=roh[:], in0=i128_sb[:],
                            scalar1=dloc_sb[:, c:c + 1], scalar2=None,
                            op0=mybir.AluOpType.is_equal,
                        )
                        ohs[c] = (loh, roh)

            def emit_abuild(k):
                dw = DWORDER[k]
                pa = ppool.tile([128, OC], F32, tag="pa", bufs=2, name=f"pa{dw}")
                for gw in range(GW):
                    ci = dw * GW + gw
                    nch_c = cellch[ci]
                    for i in range(nch_c):
                        c = cellbase[ci] + i
                        loh, roh = ohs.pop(c)
                        nc.tensor.matmul(
                            pa[:, gw * 128:(gw + 1) * 128],
                            loh[:], roh[:],
                            start=(i == 0), stop=(i == nch_c - 1),
                        )
                a_dw = padw.tile([128, OC], BF16, tag="adw", bufs=PO_AT + 6,
                                 name=f"adw{dw}")
                nc.scalar.copy(a_dw[:], pa[:])
                adws[dw] = a_dw

            def emit_po(k):
                dw = DWORDER[k]
                half, pos = k // 16, k % 16
                po = ppool.tile([128, OC], F32, tag="po", bufs=2, name=f"po{dw}")
                for gw in range(GW):
                    nc.tensor.matmul(
                        po[:],
                        adws[dw][:, gw * 128:(gw + 1) * 128],
                        xwf[gw][:],
                        start=(gw == 0), stop=(gw == GW - 1),
                    )
                if pos % 2 == 0:
                    stage[half] = wpool.tile([128, 2 * OC], F16, tag="stg",
                                             bufs=4, name=f"stg{k // 2}")
                nc.scalar.copy(
                    stage[half][:, (pos % 2) * OC:(pos % 2 + 1) * OC], po[:]
                )
                if pos % 2 == 1:
                    nc.sync.dma_start(prts[half][pos // 2], stage[half][:])

            def emit_rs(half):
                nc.gpsimd.collective_compute(
                    "ReduceScatter",
                    mybir.AluOpType.add,
                    replica_groups=[list(range(CORES))],
                    ins=[prts[half][:].opt()],
                    outs=[rsouts[half][:].opt()],
                )

            def emit_post(half):
                # post-scatter (g-scale + bias) runs entirely on PE so no
                # worker engine has RS-dependent work (which would poison
                # the vector-clock fences of the second-half pipeline):
                #   psum = diag(g)^T @ rs  +  ones^T @ bias
                rs_sb = wpool.tile([128, 2 * OC], F16, tag="rs", bufs=2,
                                   name=f"rs{half}")
                nc.sync.dma_start(rs_sb[:, 0:OC], rsouts[half][:, 0, :])
                nc.sync.dma_start(rs_sb[:, OC:], rsouts[half][:, 1, :])
                ot = wpool.tile([128, 2 * OC], F32, tag="osb", bufs=2,
                                name=f"ot{half}")
                for j in range(2):
                    pp = ppool.tile([128, OC], F32, tag="pxw", bufs=4,
                                    name=f"ppost{half}{j}")
                    nc.tensor.matmul(
                        pp[:],
                        onesr[:],
                        bias16[:],
                        start=True, stop=False,
                    )
                    nc.tensor.matmul(
                        pp[:],
                        dgt[2 * half + j][:],
                        rs_sb[:, j * OC:(j + 1) * OC],
                        start=False, stop=True,
                    )
                    nc.scalar.copy(ot[:, j * OC:(j + 1) * OC], pp[:])
                    nc.sync.dma_start(
                        out[:, 2 * half + j, :], ot[:, j * OC:(j + 1) * OC]
                    )

            next_po = 0
            for k in range(16):
                emit_ohs(k)
                if k >= 2 and (k - 2) % 4 == 0:
                    emit_ohs(16 + (k - 2) // 4)
                if k == 0:
                    f_sb = deg_scale(ssrc_sb, GW, padr)   # per local gene row
                emit_abuild(k)
                if k in (4, 6, 8, 10):
                    emit_xw((k - 4) // 2)
                if k >= PO_AT:
                    while next_po <= k - PO_LAG:
                        emit_po(next_po)
                        next_po += 1
            while next_po < 16:
                emit_po(next_po)
                next_po += 1
            for kk in range(20, 32):
                emit_ohs(kk)
            emit_g_diag()
            emit_rs(0)
            for k in range(16, len(DWORDER)):
                emit_abuild(k)
                while next_po <= k - PO_LAG:
                    emit_po(next_po)
                    next_po += 1
            while next_po < len(DWORDER):
                emit_po(next_po)
                next_po += 1
            tc.strict_bb_all_engine_barrier()
            emit_rs(1)
            emit_post(0)
            emit_post(1)

    nc.finalize()
    return nc


def make_in_maps(x, weight, bias, edge_index):
    """Host-side sharding/layout only: no arithmetic on tensor values."""
    global _WCH, _PADR, _PADC, _CELLCH
    x = np.asarray(x, dtype=np.float32)
    weight = np.asarray(weight, dtype=np.float32)
    bias = np.asarray(bias, dtype=np.float32)
    ei = np.asarray(edge_index)
    s_all = ei[0].astype(np.int64)
    d_all = ei[1].astype(np.int64)
    assert s_all.min() >= 0 and s_all.max() < CORES * GSH, "src ids out of range"
    assert d_all.min() >= 0 and d_all.max() < DWT * 128, "dst ids out of range"

    # layout knobs must cover the actual distribution
    max_cell = 0
    rmax = 0
    for c in range(CORES):
        m = (s_all >> 9) == c
        sl = s_all[m] - c * GSH
        d = d_all[m]
        cell = (d >> 7) * GW + (sl >> 7)
        max_cell = max(max_cell, int(np.bincount(cell, minlength=DWT * GW).max()))
        if sl.size:
            rmax = max(rmax, int(np.bincount(sl, minlength=GSH).max()))
    cmax = int(np.bincount(d_all, minlength=DWT * 128).max())
    wch = max(_WCH, -(-max_cell // 128))
    padr = max(_PADR, -(-rmax // 16) * 16)
    padc = max(_PADC, -(-cmax // 16) * 16)
    cell_max = np.zeros(DWT * GW, dtype=np.int64)
    for c in range(CORES):
        m = (s_all >> 9) == c
        sl = s_all[m] - c * GSH
        d = d_all[m]
        cell = (d >> 7) * GW + (sl >> 7)
        cell_max = np.maximum(cell_max, np.bincount(cell, minlength=DWT * GW))
    cellch = tuple(int(x) for x in np.maximum(1, -(-cell_max // 128)))
    cellbase = np.zeros(DWT * GW, dtype=np.int64)
    cellbase[1:] = np.cumsum(cellch)[:-1]
    _WCH, _PADR, _PADC, _CELLCH = wch, padr, padc, cellch
    nch = int(cellbase[-1] + cellch[-1])

    brep = np.ascontiguousarray(np.tile(bias[None, :], (128, 1)).astype(np.float32))
    i128 = np.ascontiguousarray(
        np.tile(np.arange(128, dtype=np.float16)[None, :], (128, 1))
    )
    pcol = np.ascontiguousarray(np.arange(128, dtype=np.float32)[:, None])
    w_t = np.ascontiguousarray(
        weight.reshape(8, 128, OC).transpose(1, 0, 2).astype(np.float32)
    )

    in_maps = []
    for c in range(CORES):
        m = (s_all >> 9) == c
        sl = s_all[m] - c * GSH
        d = d_all[m]

        sloc_lin = np.full(nch * 128, -1.0, dtype=np.float32)
        dloc_lin = np.full(nch * 128, -1.0, dtype=np.float32)
        cell = (d >> 7) * GW + (sl >> 7)
        o = np.argsort(cell, kind="stable")
        cell_o = cell[o]
        sl_o = sl[o]
        d_o2 = d[o]
        starts = np.searchsorted(cell_o, np.arange(DWT * GW))
        counts = np.bincount(cell_o, minlength=DWT * GW)
        for ci in range(DWT * GW):
            n = int(counts[ci])
            if n == 0:
                continue
            pos = int(starts[ci])
            base = int(cellbase[ci]) * 128
            sloc_lin[base:base + n] = (sl_o[pos:pos + n] & 127).astype(np.float32)
            dloc_lin[base:base + n] = (d_o2[pos:pos + n] & 127).astype(np.float32)
        sloc_t = np.ascontiguousarray(sloc_lin.reshape(nch, 128).T)
        dloc_t = np.ascontiguousarray(dloc_lin.reshape(nch, 128).T)

        # row-degree bins: (p = sl%128, w = sl>>7) holds value p (f16-exact)
        ssrc = np.full((128, GW * padr), -1.0, dtype=np.float16)
        sl_s = np.sort(sl, kind="stable")
        rank = np.arange(len(sl_s)) - np.searchsorted(sl_s, sl_s)
        ssrc[sl_s & 127, (sl_s >> 7) * padr + rank] = (sl_s & 127).astype(np.float16)

        # col-degree bins for this core's own dst window [512c, 512c+512)
        dm = (d_all >> 9) == c
        dl = d_all[dm] - c * 512
        sdst = np.full((128, 4 * padc), -1.0, dtype=np.float16)
        dl_s = np.sort(dl, kind="stable")
        rankd = np.arange(len(dl_s)) - np.searchsorted(dl_s, dl_s)
        sdst[dl_s & 127, (dl_s >> 7) * padc + rankd] = (dl_s & 127).astype(np.float16)

        xs = x[c * GSH:(c + 1) * GSH, :]  # [512 g, 1024 ic]
        xT_t = np.ascontiguousarray(
            xs.T.reshape(8, 128, GSH).transpose(1, 0, 2).astype(np.float32)
        )

        in_maps.append(
            {
                "xT": xT_t,
                "w": w_t,
                "brep": brep,
                "i128": i128,
                "pcol": pcol,
                "sloc": sloc_t,
                "dloc": dloc_t,
                "ssrc": np.ascontiguousarray(ssrc),
                "sdst": np.ascontiguousarray(sdst),
            }
        )
    return in_maps


_NC = None
_NC_KEY = None


def kernel(x, weight, bias, edge_index, **run_kwargs):
    global _NC, _NC_KEY
    from concourse.bass_utils import run_bass_kernel_spmd

    in_maps = make_in_maps(x, weight, bias, edge_index)
    key = (_WCH, _PADR, _PADC, _CELLCH)
    if _NC is None or _NC_KEY != key:
        _NC = build_nc()
        _NC_KEY = key
    res = run_bass_kernel_spmd(_NC, in_maps, core_ids=list(range(CORES)), **run_kwargs)
    outs = res.results if hasattr(res, "results") else res
    full = np.empty((CORES * GSH, OC), dtype=np.float32)
    for c in range(CORES):
        o = outs[c]["out"]  # [128, 4, OC]; row = 512c + 128*j + p
        full[c * GSH:(c + 1) * GSH] = o.transpose(1, 0, 2).reshape(GSH, OC)
    full = full[:ND]
    if run_kwargs:
        return full, res
    return full
